# revision 14
# baseline (speedup 1.0000x reference)
"""MultiHeadAttention (qk-LayerNorm + RoPE) Trainium2 kernel, 8 NeuronCores.

Sharding: batch (4) x head-group (2x8 heads). Core c handles batch c//2,
heads 8*(c%2) .. 8*(c%2)+7. Each core computes QKV projections for its
batch restricted to its head group, per-head LayerNorm + rotary embedding,
attention, and a partial output projection over its 512 context channels.
A pairwise ReduceScatter sums the two partial o_proj results per batch and
leaves each core with half the rows; the host concatenates.

All matmuls run as float32r (tf32-like) on the PE. Attention is computed
with scores transposed ([s, t] layout) so softmax normalization can be
deferred: ctx_unnorm and sum-of-exp come from one matmul with a ones row
appended to V, and the per-token reciprocal is broadcast across partitions
with a K=1 matmul.
"""
import sys

for _p in ("/opt/trn_rl_repo", "/root/.axon_site", "/root/.axon_site/_ro/trn_rl_repo",
           "/root/.axon_site/_ro/pypackages"):
    if _p not in sys.path:
        sys.path.append(_p)

import numpy as np

import concourse.bass as bass
import concourse.tile as tile
from concourse import bacc, mybir
from concourse.bass_utils import run_bass_kernel_spmd
from concourse.masks import make_identity

F32 = mybir.dt.float32
F32R = mybir.dt.float32r
P = 128
B, L, C, H, D = 4, 1024, 1024, 16, 64
HC = 8          # heads per core
CG = HC * D     # 512 context channels per core
NT = L // P     # 8 token tiles
NCK = C // P    # 8 contraction tiles
THETA = 50000.0
EPS = 1e-5

_NC_CACHE = {}


def _build_nc():
    nc = bacc.Bacc("TRN2", target_bir_lowering=False, debug=False, num_devices=8)

    xT_d = nc.dram_tensor("xT", [C, L], F32, kind="ExternalInput")
    wqT_d = nc.dram_tensor("wqT", [C, CG], F32, kind="ExternalInput")
    wkT_d = nc.dram_tensor("wkT", [C, CG], F32, kind="ExternalInput")
    wvT_d = nc.dram_tensor("wvT", [C, CG], F32, kind="ExternalInput")
    woT_d = nc.dram_tensor("woT", [D, HC, C], F32, kind="ExternalInput")
    aq_d = nc.dram_tensor("aq", [L, D], F32, kind="ExternalInput")
    bq_d = nc.dram_tensor("bq", [L, D], F32, kind="ExternalInput")
    ak_d = nc.dram_tensor("ak", [L, D], F32, kind="ExternalInput")
    bk_d = nc.dram_tensor("bk", [L, D], F32, kind="ExternalInput")
    ones_d = nc.dram_tensor("ones64", [1, D], F32, kind="ExternalInput")
    out_d = nc.dram_tensor("out", [L // 2, C], F32, kind="ExternalOutput")

    with tile.TileContext(nc) as tc:
        with (
            tc.tile_pool(name="const", bufs=1) as constp,
            tc.tile_pool(name="w", bufs=1) as wpool,
            tc.tile_pool(name="big", bufs=1) as bigp,
            tc.tile_pool(name="xt", bufs=2) as xtp,
            tc.tile_pool(name="sq", bufs=1) as sqp,
            tc.tile_pool(name="scr", bufs=2) as scrp,
            tc.tile_pool(name="rope", bufs=2) as ropep,
            tc.tile_pool(name="stat", bufs=1) as statp,
            tc.tile_pool(name="exp", bufs=2) as expp,
            tc.tile_pool(name="fin", bufs=2) as finp,
            tc.tile_pool(name="dram", bufs=1, space="DRAM") as dram,
        ):
            ident = constp.tile([P, P], F32)
            make_identity(nc, ident)
            eps_t = constp.tile([P, 1], F32)
            nc.vector.memset(eps_t[:], EPS)
            ones_row = constp.tile([65, D], F32R)
            nc.sync.dma_start(ones_row[64:65, :], ones_d.ap().bitcast(F32R))

            # per-ck weight tiles so the first matmuls start after ~1.5MB of DMA
            wq_t, wk_t, wv_t = [], [], []
            for lst, nm, d_ in ((wq_t, "wq", wqT_d), (wk_t, "wk", wkT_d),
                                (wv_t, "wv", wvT_d)):
                for ck in range(NCK):
                    t_ = wpool.tile([P, CG], F32R, tag=f"{nm}{ck}", name=f"{nm}{ck}")
                    nc.sync.dma_start(
                        t_[:],
                        d_.ap().rearrange("(k p) o -> p k o", p=P)[:, ck, :].bitcast(F32R))
                    lst.append(t_)

            aq_t = constp.tile([P, NT, D], F32)
            nc.sync.dma_start(aq_t[:], aq_d.ap().rearrange("(t p) d -> p t d", p=P))
            bq_t = constp.tile([P, NT, D], F32)
            nc.sync.dma_start(bq_t[:], bq_d.ap().rearrange("(t p) d -> p t d", p=P))
            ak_t = constp.tile([P, NT, D], F32)
            nc.sync.dma_start(ak_t[:], ak_d.ap().rearrange("(t p) d -> p t d", p=P))
            bk_t = constp.tile([P, NT, D], F32)
            nc.sync.dma_start(bk_t[:], bk_d.ap().rearrange("(t p) d -> p t d", p=P))

            # v with a ones column appended per head: [s_tile, j, head, 65]
            v_sb = bigp.tile([P, NT, HC, D + 1], F32R)
            nc.sync.dma_start(
                v_sb[:, :, :, D:D + 1].rearrange("p t h o -> p (t h) o"),
                ones_d.ap()[0:1, 0:1].rearrange("a b -> a b ()").to_broadcast(
                    (P, NT * HC, 1)).bitcast(F32R),
            )
            qT_pack = bigp.tile([P, HC // 2, L], F32R)
            kT_pack = bigp.tile([P, HC // 2, L], F32R)
            ctxT = bigp.tile([D, HC, L], F32R)

            # ---------------- Phase 1: QKV + LN + RoPE + transpose ----------
            with tc.tile_pool(name="ps1", bufs=2, space="PSUM") as ps1, \
                 tc.tile_pool(name="pst", bufs=2, space="PSUM") as pst:
                for ti in range(NT):
                    xt = xtp.tile([P, NCK, P], F32R)
                    nc.gpsimd.dma_start(
                        xt[:],
                        xT_d.ap().rearrange("(k p) t -> p k t", p=P)[:, :, bass.ts(ti, P)].bitcast(F32R),
                    )
                    psq = ps1.tile([P, CG], F32, tag="psq")
                    psk = ps1.tile([P, CG], F32, tag="psk")
                    psv = ps1.tile([P, CG], F32, tag="psv")
                    for ps_, w_ in ((psq, wq_t), (psk, wk_t), (psv, wv_t)):
                        for ck in range(NCK):
                            nc.tensor.matmul(ps_[:], xt[:, ck, :], w_[ck][:],
                                             start=(ck == 0), stop=(ck == NCK - 1))

                    # v straight to SBUF (rounded to f32r); ACT engine to keep DVE free
                    nc.scalar.copy(
                        v_sb[:, ti, :, 0:D],
                        psv[:].rearrange("p (h d) -> p h d", d=D))

                    # LN stats for q and k: sums and sums of squares
                    stats = statp.tile([P, 4, HC], F32)
                    for i, ps_ in enumerate((psq, psk)):
                        nc.vector.reduce_sum(
                            stats[:, 2 * i, :], ps_[:].rearrange("p (h d) -> p h d", d=D),
                            axis=mybir.AxisListType.X)
                        sq = sqp.tile([P, CG], F32)
                        nc.scalar.square(sq[:], ps_[:])
                        nc.vector.reduce_sum(
                            stats[:, 2 * i + 1, :], sq[:].rearrange("p (h d) -> p h d", d=D),
                            axis=mybir.AxisListType.X)
                    mus = statp.tile([P, 2, HC], F32)
                    nc.vector.tensor_scalar_mul(mus[:], stats[:, 0::2, :], 1.0 / D)
                    ms2 = statp.tile([P, 2, HC], F32)
                    nc.vector.tensor_scalar_mul(ms2[:], stats[:, 1::2, :], 1.0 / D)
                    var = statp.tile([P, 2, HC], F32)
                    nc.vector.tensor_mul(var[:], mus[:], mus[:])
                    nc.vector.tensor_sub(var[:], ms2[:], var[:])
                    std = statp.tile([P, 2, HC], F32)
                    nc.scalar.activation(std[:], var[:], mybir.ActivationFunctionType.Sqrt,
                                         bias=eps_t[:])
                    invstd = statp.tile([P, 2, HC], F32)
                    nc.vector.reciprocal(invstd[:], std[:])
                    shift = statp.tile([P, 2, HC], F32)
                    nc.vector.tensor_mul(shift[:], mus[:], invstd[:])

                    for i, (ps_, a_t, b_t, dstpack) in enumerate(
                            ((psq, aq_t, bq_t, qT_pack), (psk, ak_t, bk_t, kT_pack))):
                        inv_b = invstd[:, i, :].rearrange("p h -> p h ()").to_broadcast((P, HC, D))
                        sh_b = shift[:, i, :].rearrange("p h -> p h ()").to_broadcast((P, HC, D))
                        a_b = a_t[:, ti, :].rearrange("p d -> p () d").to_broadcast((P, HC, D))
                        t1 = scrp.tile([P, HC, D], F32, tag="t1")
                        nc.vector.tensor_mul(t1[:], ps_[:].rearrange("p (h d) -> p h d", d=D), inv_b)
                        nc.vector.tensor_sub(t1[:], t1[:], sh_b)
                        rope = ropep.tile([P, HC, D], F32, tag=f"rope{i}")
                        nc.vector.tensor_mul(rope[:], t1[:], a_b)
                        r2 = scrp.tile([P, HC, D], F32, tag="r2")
                        h_ = D // 2
                        nc.vector.tensor_mul(
                            r2[:, :, 0:h_], t1[:, :, h_:D],
                            b_t[:, ti, 0:h_].rearrange("p d -> p () d").to_broadcast((P, HC, h_)))
                        nc.vector.tensor_mul(
                            r2[:, :, h_:D], t1[:, :, 0:h_],
                            b_t[:, ti, h_:D].rearrange("p d -> p () d").to_broadcast((P, HC, h_)))
                        nc.vector.tensor_add(rope[:], rope[:], r2[:])
                        for pr in range(HC // 2):
                            ps_t = pst.tile([P, P], F32)
                            nc.tensor.transpose(
                                ps_t[:],
                                rope[:, 2 * pr:2 * pr + 2, :].rearrange("p h d -> p (h d)"),
                                ident[:])
                            nc.scalar.copy(dstpack[:, pr, bass.ts(ti, P)], ps_t[:])

            # ---------------- Phase 2: attention per head -------------------
            with tc.tile_pool(name="pss", bufs=1, space="PSUM") as pssp, \
                 tc.tile_pool(name="psc", bufs=2, space="PSUM") as pscp, \
                 tc.tile_pool(name="psr", bufs=1, space="PSUM") as psrp:
                for h in range(HC):
                    pr, sub = h // 2, h % 2
                    lo, hi = D * sub, D * sub + D
                    psc = pscp.tile([D + 1, L], F32)
                    for j in range(NT):
                        pss = pssp.tile([P, L], F32)
                        for m in range(2):
                            nc.tensor.matmul(
                                pss[:, bass.ts(m, 512)],
                                kT_pack[lo:hi, pr, bass.ts(j, P)],
                                qT_pack[lo:hi, pr, bass.ts(m, 512)],
                                start=True, stop=True)
                        expT = expp.tile([P, L], F32R)
                        nc.scalar.activation(expT[:], pss[:],
                                             mybir.ActivationFunctionType.Exp,
                                             scale=float(D) ** -0.5)
                        for m in range(2):
                            nc.tensor.matmul(
                                psc[:, bass.ts(m, 512)],
                                v_sb[:, j, h, :],
                                expT[:, bass.ts(m, 512)],
                                start=(j == 0), stop=(j == NT - 1))
                    recip = finp.tile([D + 1, L], F32R, tag="recip")
                    with nc.allow_low_precision(reason="f32r rounding for rb matmul"):
                        nc.vector.reciprocal(recip[D:D + 1, :], psc[D:D + 1, :])
                    ps_rb = psrp.tile([D, L], F32, name="ps_rb")
                    for m in range(2):
                        nc.tensor.matmul(
                            ps_rb[:, bass.ts(m, 512)],
                            ones_row[64:65, :],
                            recip[D:D + 1, bass.ts(m, 512)],
                            start=True, stop=True)
                    rb_sb = finp.tile([D, L], F32, tag="rb")
                    nc.vector.tensor_copy(rb_sb[:], ps_rb[:])
                    nc.vector.tensor_mul(ctxT[:, h, :], psc[0:D, :], rb_sb[:])

            # ---------------- Phase 3: output projection --------------------
            # wo reuses the per-ck wq slots (dead after phase 1)
            wo_l = []
            for h in range(HC):
                wo_h = wpool.tile([D, C], F32R, tag=f"wq{h}", name=f"wo{h}")
                nc.sync.dma_start(wo_h[:], woT_d.ap()[:, h, :].bitcast(F32R))
                wo_l.append(wo_h)

            bounce_in = [dram.tile([L // 2, C], F32, tag=f"bin{i}", name=f"bin{i}")
                         for i in range(2)]
            bounce_out = [dram.tile([L // 4, C], F32, tag=f"bout{i}", name=f"bout{i}")
                         for i in range(2)]

            def emit_rs(half):
                nc.gpsimd.collective_compute(
                    "ReduceScatter",
                    mybir.AluOpType.add,
                    replica_groups=[[0, 1], [2, 3], [4, 5], [6, 7]],
                    ins=[bounce_in[half][:].opt()],
                    outs=[bounce_out[half][:].opt()],
                )
                nc.sync.dma_start(out_d.ap()[bass.ts(half, L // 4), :],
                                  bounce_out[half][:])

            with tc.tile_pool(name="pso", bufs=2, space="PSUM") as psop:
                for ti in range(NT):
                    pso = psop.tile([P, C], F32)
                    for m in range(2):
                        for h in range(HC):
                            nc.tensor.matmul(
                                pso[:, bass.ts(m, 512)],
                                ctxT[:, h, bass.ts(ti, P)],
                                wo_l[h][:, bass.ts(m, 512)],
                                start=(h == 0), stop=(h == HC - 1))
                    out_sb = finp.tile([P, C], F32, tag="out", bufs=1)
                    nc.vector.tensor_copy(out_sb[:], pso[:])
                    nc.sync.dma_start(bounce_in[ti // 4][bass.ts(ti % 4, P), :], out_sb[:])
                    if ti == NT // 2 - 1:
                        emit_rs(0)
                emit_rs(1)

    nc.compile()
    return nc


def _rope_tables(w, b):
    """A[t,d], B[t,d] with the rotate-half sign folded into B."""
    inv_freq = 1.0 / THETA ** (np.arange(0, D, 2, dtype=np.float64) / D)
    freqs = np.arange(L, dtype=np.float64)[:, None] * inv_freq[None, :]
    freqs = np.concatenate([freqs, freqs], axis=1)           # [L, D]
    cos, sin = np.cos(freqs), np.sin(freqs)
    w = w.astype(np.float64)
    w_rot = np.concatenate([w[D // 2:], w[:D // 2]])
    sgn = np.concatenate([-np.ones(D // 2), np.ones(D // 2)])
    A = (cos * w[None, :]).astype(np.float32)
    Bt = (sin * w_rot[None, :] * sgn[None, :]).astype(np.float32)
    if np.any(b != 0):
        raise NotImplementedError("nonzero qk-norm bias not supported")
    return A, Bt


def kernel(**inputs):
    x = np.asarray(inputs["q"], dtype=np.float32)
    Wq = np.asarray(inputs["Wq"], dtype=np.float32)
    Wk = np.asarray(inputs["Wk"], dtype=np.float32)
    Wv = np.asarray(inputs["Wv"], dtype=np.float32)
    Wo = np.asarray(inputs["Wo"], dtype=np.float32)
    bo = np.asarray(inputs["bo"], dtype=np.float32)
    assert not np.any(bo != 0), "nonzero output bias not supported"

    Aq, Bq = _rope_tables(np.asarray(inputs["qn_w"], np.float32),
                          np.asarray(inputs["qn_b"], np.float32))
    Ak, Bk = _rope_tables(np.asarray(inputs["kn_w"], np.float32),
                          np.asarray(inputs["kn_b"], np.float32))
    ones64 = np.ones((1, D), dtype=np.float32)
    WoT = np.ascontiguousarray(Wo.T)                          # [C(c'), C(o)]

    if "nc" not in _NC_CACHE:
        _NC_CACHE["nc"] = _build_nc()
    nc = _NC_CACHE["nc"]

    in_maps = []
    for c in range(8):
        b_, g = c // 2, c % 2
        sl = slice(g * CG, (g + 1) * CG)
        in_maps.append({
            "xT": np.ascontiguousarray(x[b_].T),
            "wqT": np.ascontiguousarray(Wq[sl, :].T),
            "wkT": np.ascontiguousarray(Wk[sl, :].T),
            "wvT": np.ascontiguousarray(Wv[sl, :].T),
            "woT": np.ascontiguousarray(
                WoT[sl, :].reshape(HC, D, C).transpose(1, 0, 2)),
            "aq": Aq, "bq": Bq, "ak": Ak, "bk": Bk,
            "ones64": ones64,
        })

    res = run_bass_kernel_spmd(nc, in_maps, core_ids=list(range(8)))
    # two half-ReduceScatters: each core's "out" holds [rank's quarter of rows
    # 0:512 ; rank's quarter of rows 512:1024]
    Q = L // 4
    out = np.empty((B, L, C), dtype=np.float32)
    for b_ in range(B):
        ev, od = res.results[2 * b_]["out"], res.results[2 * b_ + 1]["out"]
        out[b_, 0 * Q:1 * Q] = ev[0:Q]
        out[b_, 1 * Q:2 * Q] = od[0:Q]
        out[b_, 2 * Q:3 * Q] = ev[Q:2 * Q]
        out[b_, 3 * Q:4 * Q] = od[Q:2 * Q]
    return out


# revision 15
# speedup vs baseline: 1.0912x; 1.0912x over previous
"""MultiHeadAttention (qk-LayerNorm + RoPE) Trainium2 kernel, 8 NeuronCores.

Sharding: batch (4) x head-group (2x8 heads). Core c handles batch c//2,
heads 8*(c%2) .. 8*(c%2)+7. Each core computes QKV projections for its
batch restricted to its head group, per-head LayerNorm + rotary embedding,
attention, and a partial output projection over its 512 context channels.
A pairwise ReduceScatter sums the two partial o_proj results per batch and
leaves each core with half the rows; the host concatenates.

All matmuls run as float32r (tf32-like) on the PE. Attention is computed
with scores transposed ([s, t] layout) so softmax normalization can be
deferred: ctx_unnorm and sum-of-exp come from one matmul with a ones row
appended to V, and the per-token reciprocal is broadcast across partitions
with a K=1 matmul.
"""
import sys

for _p in ("/opt/trn_rl_repo", "/root/.axon_site", "/root/.axon_site/_ro/trn_rl_repo",
           "/root/.axon_site/_ro/pypackages"):
    if _p not in sys.path:
        sys.path.append(_p)

import numpy as np

import concourse.bass as bass
import concourse.tile as tile
from concourse import bacc, mybir
from concourse.bass_utils import run_bass_kernel_spmd
from concourse.masks import make_identity

F32 = mybir.dt.float32
F32R = mybir.dt.float32r
P = 128
B, L, C, H, D = 4, 1024, 1024, 16, 64
HC = 8          # heads per core
CG = HC * D     # 512 context channels per core
NT = L // P     # 8 token tiles
NCK = C // P    # 8 contraction tiles
THETA = 50000.0
EPS = 1e-5

_NC_CACHE = {}


def _build_nc():
    nc = bacc.Bacc("TRN2", target_bir_lowering=False, debug=False, num_devices=8)

    xT_d = nc.dram_tensor("xT", [C, L], F32, kind="ExternalInput")
    wqT_d = nc.dram_tensor("wqT", [C, CG], F32, kind="ExternalInput")
    wkT_d = nc.dram_tensor("wkT", [C, CG], F32, kind="ExternalInput")
    wvT_d = nc.dram_tensor("wvT", [C, CG], F32, kind="ExternalInput")
    woT_d = nc.dram_tensor("woT", [D, HC, C], F32, kind="ExternalInput")
    aq_d = nc.dram_tensor("aq", [L, D], F32, kind="ExternalInput")
    bq_d = nc.dram_tensor("bq", [L, D], F32, kind="ExternalInput")
    ak_d = nc.dram_tensor("ak", [L, D], F32, kind="ExternalInput")
    bk_d = nc.dram_tensor("bk", [L, D], F32, kind="ExternalInput")
    ones_d = nc.dram_tensor("ones64", [1, D], F32, kind="ExternalInput")
    out_d = nc.dram_tensor("out", [L // 2, C], F32, kind="ExternalOutput")

    with tile.TileContext(nc) as tc:
        with (
            tc.tile_pool(name="const", bufs=1) as constp,
            tc.tile_pool(name="w", bufs=1) as wpool,
            tc.tile_pool(name="big", bufs=1) as bigp,
            tc.tile_pool(name="xt", bufs=2) as xtp,
            tc.tile_pool(name="sq", bufs=1) as sqp,
            tc.tile_pool(name="scr", bufs=2) as scrp,
            tc.tile_pool(name="rope", bufs=2) as ropep,
            tc.tile_pool(name="stat", bufs=2) as statp,
            tc.tile_pool(name="exp", bufs=2) as expp,
            tc.tile_pool(name="fin", bufs=1) as finp,
            tc.tile_pool(name="dram", bufs=1, space="DRAM") as dram,
        ):
            ident = constp.tile([P, P], F32)
            make_identity(nc, ident)
            eps_t = constp.tile([P, 1], F32)
            nc.vector.memset(eps_t[:], EPS)
            ones_row = constp.tile([65, D], F32R)
            nc.sync.dma_start(ones_row[64:65, :], ones_d.ap().bitcast(F32R))

            aq_t = constp.tile([P, NT, D], F32)
            nc.sync.dma_start(aq_t[:], aq_d.ap().rearrange("(t p) d -> p t d", p=P))
            bq_t = constp.tile([P, NT, D], F32)
            nc.sync.dma_start(bq_t[:], bq_d.ap().rearrange("(t p) d -> p t d", p=P))
            ak_t = constp.tile([P, NT, D], F32)
            nc.sync.dma_start(ak_t[:], ak_d.ap().rearrange("(t p) d -> p t d", p=P))
            bk_t = constp.tile([P, NT, D], F32)
            nc.sync.dma_start(bk_t[:], bk_d.ap().rearrange("(t p) d -> p t d", p=P))

            # per-ck weight tiles so the first matmuls start after ~1.5MB of DMA
            wq_t, wk_t, wv_t = [], [], []
            for ck in range(NCK):
                for lst, nm, d_ in ((wq_t, "wq", wqT_d), (wk_t, "wk", wkT_d),
                                    (wv_t, "wv", wvT_d)):
                    t_ = wpool.tile([P, CG], F32R, tag=f"{nm}{ck}", name=f"{nm}{ck}")
                    nc.sync.dma_start(
                        t_[:],
                        d_.ap().rearrange("(k p) o -> p k o", p=P)[:, ck, :].bitcast(F32R))
                    lst.append(t_)

            # v with a ones column appended per head: [s_tile, j, head, 65]
            v_sb = bigp.tile([P, NT, HC, D + 1], F32R)
            nc.sync.dma_start(
                v_sb[:, :, :, D:D + 1].rearrange("p t h o -> p (t h) o"),
                ones_d.ap()[0:1, 0:1].rearrange("a b -> a b ()").to_broadcast(
                    (P, NT * HC, 1)).bitcast(F32R),
            )
            qT_pack = bigp.tile([P, HC // 2, L], F32R)
            kT_pack = bigp.tile([P, HC // 2, L], F32R)
            ctxT = bigp.tile([D, HC, L], F32R)

            # ---------------- Phase 1: QKV + LN + RoPE + transpose ----------
            with tc.tile_pool(name="ps1", bufs=2, space="PSUM") as ps1, \
                 tc.tile_pool(name="pst", bufs=2, space="PSUM") as pst:
                for ti in range(NT):
                    xt = xtp.tile([P, NCK, P], F32R)
                    nc.sync.dma_start(
                        xt[:],
                        xT_d.ap().rearrange("(k p) t -> p k t", p=P)[:, :, bass.ts(ti, P)].bitcast(F32R),
                    )
                    psq = ps1.tile([P, CG], F32, tag="psq")
                    psk = ps1.tile([P, CG], F32, tag="psk")
                    psv = ps1.tile([P, CG], F32, tag="psv")
                    for ps_, w_ in ((psq, wq_t), (psk, wk_t), (psv, wv_t)):
                        for ck in range(NCK):
                            nc.tensor.matmul(ps_[:], xt[:, ck, :], w_[ck][:],
                                             start=(ck == 0), stop=(ck == NCK - 1))

                    # v straight to SBUF (rounded to f32r); ACT engine to keep DVE free
                    nc.scalar.copy(
                        v_sb[:, ti, :, 0:D],
                        psv[:].rearrange("p (h d) -> p h d", d=D))

                    # LN stats for q and k: sums and sums of squares
                    stats = statp.tile([P, 4, HC], F32)
                    for i, ps_ in enumerate((psq, psk)):
                        nc.vector.reduce_sum(
                            stats[:, 2 * i, :], ps_[:].rearrange("p (h d) -> p h d", d=D),
                            axis=mybir.AxisListType.X)
                        sq = sqp.tile([P, CG], F32)
                        nc.scalar.square(sq[:], ps_[:])
                        nc.vector.reduce_sum(
                            stats[:, 2 * i + 1, :], sq[:].rearrange("p (h d) -> p h d", d=D),
                            axis=mybir.AxisListType.X)
                    mus = statp.tile([P, 2, HC], F32)
                    nc.vector.tensor_scalar_mul(mus[:], stats[:, 0::2, :], 1.0 / D)
                    ms2 = statp.tile([P, 2, HC], F32)
                    nc.vector.tensor_scalar_mul(ms2[:], stats[:, 1::2, :], 1.0 / D)
                    var = statp.tile([P, 2, HC], F32)
                    nc.vector.tensor_mul(var[:], mus[:], mus[:])
                    nc.vector.tensor_sub(var[:], ms2[:], var[:])
                    std = statp.tile([P, 2, HC], F32)
                    nc.scalar.activation(std[:], var[:], mybir.ActivationFunctionType.Sqrt,
                                         bias=eps_t[:])
                    invstd = statp.tile([P, 2, HC], F32)
                    nc.vector.reciprocal(invstd[:], std[:])
                    shift = statp.tile([P, 2, HC], F32)
                    nc.vector.tensor_mul(shift[:], mus[:], invstd[:])

                    for i, (ps_, a_t, b_t, dstpack) in enumerate(
                            ((psq, aq_t, bq_t, qT_pack), (psk, ak_t, bk_t, kT_pack))):
                        inv_b = invstd[:, i, :].rearrange("p h -> p h ()").to_broadcast((P, HC, D))
                        sh_b = shift[:, i, :].rearrange("p h -> p h ()").to_broadcast((P, HC, D))
                        a_b = a_t[:, ti, :].rearrange("p d -> p () d").to_broadcast((P, HC, D))
                        t1 = scrp.tile([P, HC, D], F32, tag="t1")
                        nc.vector.tensor_mul(t1[:], ps_[:].rearrange("p (h d) -> p h d", d=D), inv_b)
                        nc.vector.tensor_sub(t1[:], t1[:], sh_b)
                        rope = ropep.tile([P, HC, D], F32, tag=f"rope{i}")
                        nc.vector.tensor_mul(rope[:], t1[:], a_b)
                        r2 = scrp.tile([P, HC, D], F32, tag="r2")
                        h_ = D // 2
                        nc.vector.tensor_mul(
                            r2[:, :, 0:h_], t1[:, :, h_:D],
                            b_t[:, ti, 0:h_].rearrange("p d -> p () d").to_broadcast((P, HC, h_)))
                        nc.vector.tensor_mul(
                            r2[:, :, h_:D], t1[:, :, 0:h_],
                            b_t[:, ti, h_:D].rearrange("p d -> p () d").to_broadcast((P, HC, h_)))
                        nc.vector.tensor_add(rope[:], rope[:], r2[:])
                        for pr in range(HC // 2):
                            ps_t = pst.tile([P, P], F32)
                            nc.tensor.transpose(
                                ps_t[:],
                                rope[:, 2 * pr:2 * pr + 2, :].rearrange("p h d -> p (h d)"),
                                ident[:])
                            nc.scalar.copy(dstpack[:, pr, bass.ts(ti, P)], ps_t[:])

            # ---------------- Phase 2: attention per head -------------------
            with tc.tile_pool(name="pss", bufs=2, space="PSUM") as pssp, \
                 tc.tile_pool(name="psc", bufs=1, space="PSUM") as pscp, \
                 tc.tile_pool(name="psr", bufs=1, space="PSUM") as psrp:
                for h in range(HC):
                    pr, sub = h // 2, h % 2
                    lo, hi = D * sub, D * sub + D
                    psc = pscp.tile([D + 1, L], F32)
                    for j in range(NT):
                        pss = pssp.tile([P, L], F32)
                        for m in range(2):
                            nc.tensor.matmul(
                                pss[:, bass.ts(m, 512)],
                                kT_pack[lo:hi, pr, bass.ts(j, P)],
                                qT_pack[lo:hi, pr, bass.ts(m, 512)],
                                start=True, stop=True)
                        expT = expp.tile([P, L], F32R)
                        nc.scalar.activation(expT[:], pss[:],
                                             mybir.ActivationFunctionType.Exp,
                                             scale=float(D) ** -0.5)
                        for m in range(2):
                            nc.tensor.matmul(
                                psc[:, bass.ts(m, 512)],
                                v_sb[:, j, h, :],
                                expT[:, bass.ts(m, 512)],
                                start=(j == 0), stop=(j == NT - 1))
                    recip = finp.tile([D + 1, L], F32R, tag="recip")
                    with nc.allow_low_precision(reason="f32r rounding for rb matmul"):
                        nc.vector.reciprocal(recip[D:D + 1, :], psc[D:D + 1, :])
                    ps_rb = psrp.tile([D, L], F32)
                    for m in range(2):
                        nc.tensor.matmul(
                            ps_rb[:, bass.ts(m, 512)],
                            ones_row[64:65, :],
                            recip[D:D + 1, bass.ts(m, 512)],
                            start=True, stop=True)
                    rb_sb = finp.tile([D, L], F32, tag="rb")
                    nc.vector.tensor_copy(rb_sb[:], ps_rb[:])
                    nc.vector.tensor_mul(ctxT[:, h, :], psc[0:D, :], rb_sb[:])

            # ---------------- Phase 3: output projection --------------------
            # wo reuses the per-ck wq slots (dead after phase 1)
            wo_l = []
            for h in range(HC):
                wo_h = wpool.tile([D, C], F32R, tag=f"wq{h}", name=f"wo{h}")
                nc.sync.dma_start(wo_h[:], woT_d.ap()[:, h, :].bitcast(F32R))
                wo_l.append(wo_h)

            bounce_in = [dram.tile([L // 2, C], F32, tag=f"bin{i}", name=f"bin{i}")
                         for i in range(2)]
            bounce_out = [dram.tile([L // 4, C], F32, tag=f"bout{i}", name=f"bout{i}")
                         for i in range(2)]

            def emit_rs(half):
                nc.gpsimd.collective_compute(
                    "ReduceScatter",
                    mybir.AluOpType.add,
                    replica_groups=[[0, 1], [2, 3], [4, 5], [6, 7]],
                    ins=[bounce_in[half][:].opt()],
                    outs=[bounce_out[half][:].opt()],
                )
                nc.sync.dma_start(out_d.ap()[bass.ts(half, L // 4), :],
                                  bounce_out[half][:])

            with tc.tile_pool(name="pso", bufs=2, space="PSUM") as psop:
                for ti in range(NT):
                    pso = psop.tile([P, C], F32)
                    for m in range(2):
                        for h in range(HC):
                            nc.tensor.matmul(
                                pso[:, bass.ts(m, 512)],
                                ctxT[:, h, bass.ts(ti, P)],
                                wo_l[h][:, bass.ts(m, 512)],
                                start=(h == 0), stop=(h == HC - 1))
                    out_sb = finp.tile([P, C], F32, tag="out")
                    nc.vector.tensor_copy(out_sb[:], pso[:])
                    nc.sync.dma_start(bounce_in[ti // 4][bass.ts(ti % 4, P), :], out_sb[:])
                    if ti == NT // 2 - 1:
                        emit_rs(0)
                emit_rs(1)

    nc.compile()
    return nc


def _rope_tables(w, b):
    """A[t,d], B[t,d] with the rotate-half sign folded into B."""
    inv_freq = 1.0 / THETA ** (np.arange(0, D, 2, dtype=np.float64) / D)
    freqs = np.arange(L, dtype=np.float64)[:, None] * inv_freq[None, :]
    freqs = np.concatenate([freqs, freqs], axis=1)           # [L, D]
    cos, sin = np.cos(freqs), np.sin(freqs)
    w = w.astype(np.float64)
    w_rot = np.concatenate([w[D // 2:], w[:D // 2]])
    sgn = np.concatenate([-np.ones(D // 2), np.ones(D // 2)])
    A = (cos * w[None, :]).astype(np.float32)
    Bt = (sin * w_rot[None, :] * sgn[None, :]).astype(np.float32)
    if np.any(b != 0):
        raise NotImplementedError("nonzero qk-norm bias not supported")
    return A, Bt


def kernel(**inputs):
    x = np.asarray(inputs["q"], dtype=np.float32)
    Wq = np.asarray(inputs["Wq"], dtype=np.float32)
    Wk = np.asarray(inputs["Wk"], dtype=np.float32)
    Wv = np.asarray(inputs["Wv"], dtype=np.float32)
    Wo = np.asarray(inputs["Wo"], dtype=np.float32)
    bo = np.asarray(inputs["bo"], dtype=np.float32)
    assert not np.any(bo != 0), "nonzero output bias not supported"

    Aq, Bq = _rope_tables(np.asarray(inputs["qn_w"], np.float32),
                          np.asarray(inputs["qn_b"], np.float32))
    Ak, Bk = _rope_tables(np.asarray(inputs["kn_w"], np.float32),
                          np.asarray(inputs["kn_b"], np.float32))
    ones64 = np.ones((1, D), dtype=np.float32)
    WoT = np.ascontiguousarray(Wo.T)                          # [C(c'), C(o)]

    if "nc" not in _NC_CACHE:
        _NC_CACHE["nc"] = _build_nc()
    nc = _NC_CACHE["nc"]

    in_maps = []
    for c in range(8):
        b_, g = c // 2, c % 2
        sl = slice(g * CG, (g + 1) * CG)
        in_maps.append({
            "xT": np.ascontiguousarray(x[b_].T),
            "wqT": np.ascontiguousarray(Wq[sl, :].T),
            "wkT": np.ascontiguousarray(Wk[sl, :].T),
            "wvT": np.ascontiguousarray(Wv[sl, :].T),
            "woT": np.ascontiguousarray(
                WoT[sl, :].reshape(HC, D, C).transpose(1, 0, 2)),
            "aq": Aq, "bq": Bq, "ak": Ak, "bk": Bk,
            "ones64": ones64,
        })

    res = run_bass_kernel_spmd(nc, in_maps, core_ids=list(range(8)))
    # two half-ReduceScatters: each core's "out" holds [rank's quarter of rows
    # 0:512 ; rank's quarter of rows 512:1024]
    Q = L // 4
    out = np.empty((B, L, C), dtype=np.float32)
    for b_ in range(B):
        ev, od = res.results[2 * b_]["out"], res.results[2 * b_ + 1]["out"]
        out[b_, 0 * Q:1 * Q] = ev[0:Q]
        out[b_, 1 * Q:2 * Q] = od[0:Q]
        out[b_, 2 * Q:3 * Q] = ev[Q:2 * Q]
        out[b_, 3 * Q:4 * Q] = od[Q:2 * Q]
    return out
